# revision 34
# baseline (speedup 1.0000x reference)
"""Multi-head causal self-attention (B=2, S=2048, D=1024, H=16) on 8 TRN2 cores.

Sharding: head-parallel attention + token-parallel output projection.
Core c owns head-group c = heads {2c, 2c+1} (128 of the 1024 qkv dims).

Per core (all matmul operands bf16, fp32 PSUM accumulation):
  stage B: Q^T/K^T/V^T = (x @ W{q,k,v}[:, c-slice] + b)^T        [128, 4096]
  stage C: V^T -> V_aug [token, 65] tiles (col 64 = ones -> l row)
  stage D: per (batch, q-chunk), heads interleaved to hide ACT exp
           latency: scores^T = K^T.T @ Q^T (PE), exp (ACT, scale 1/8),
           causal mask on diagonal tiles (DVE, multiplicative),
           ctx^T accum (PE) with the ones column producing l in row 64.
  stage E: per chunk: r = 1/l (DVE reciprocal), broadcast via PE outer
           product, normalize ctx^T from PSUM -> ctxn bf16.
  stage F: DMA ctxn token-blocks to a2a_in; AllToAll (0.5 MB/batch)
           redistributes so core j holds ALL 1024 ctx dims for its 256
           tokens of the batch.
  stage G: y[tok, :] = ctx_full^T.T-tiles @ Wo + bo for the core's own
           tokens (Wo replicated). Batch 0's G is interleaved into
           batch 1's attention; only batch 1's AllToAll + G is a tail.

Host: passes x^T and weights in bf16, Wo/bo replicated; output comes
back as [512 tokens, 1024] fp32 rows per core, no transpose needed.
"""

import sys

for p in ("/opt/trn_rl_repo", "/root/.axon_site/_ro/trn_rl_repo"):
    if p not in sys.path:
        sys.path.insert(0, p)

import numpy as np
import ml_dtypes

import bass_rust
import concourse.bass as bass
import concourse.mybir as mybir
from concourse.bass_utils import run_bass_kernel_spmd
from concourse.masks import make_identity
from concourse.tile import TileContext

B, S, D = 2, 2048, 1024
H, DH = 16, 64
T = B * S              # 4096 tokens
NC = 8                 # cores
HG = D // NC           # 128 qkv dims per core (2 heads)
KT_D = D // 128        # 8 contraction tiles over d_model
TPC = S // NC          # 256 tokens per core per batch
INV_SCALE = 1.0 / float(np.sqrt(DH))  # 1/8
F32 = mybir.dt.float32
F32R = mybir.dt.float32r
BF16 = mybir.dt.bfloat16


def _split_waits(nc, max_waits=1):
    """This walrus build accepts one sync-wait per instruction; Tile sometimes
    emits more. Split extras into preceding NoOps on the same engine."""
    n = 0
    for f in nc.m.functions:
        for bb in f.blocks:
            out = []
            for inst in bb.instructions:
                si = getattr(inst, "sync_info", None)
                if si is not None and si.on_wait and len(si.on_wait) > max_waits:
                    waits = list(si.on_wait)
                    head, rest = waits[:-max_waits], waits[-max_waits:]
                    k = 0
                    while head:
                        chunk, head = head[:max_waits], head[max_waits:]
                        out.append(mybir.InstNoOp(
                            name=f"{inst.name}-wsplit-{k}", ins=[], outs=[],
                            engine=inst.engine,
                            sync_info=bass_rust.SyncInfo(on_wait=chunk, on_update=[]),
                        ))
                        k += 1
                    si.on_wait = rest
                    n += 1
                out.append(inst)
            bb.instructions = out
    return n


def build_module():
    nc = bass.Bass()

    xT = nc.dram_tensor("xT", [D, T], BF16, kind="ExternalInput")
    wq = nc.dram_tensor("wq", [D, HG], BF16, kind="ExternalInput")
    wk = nc.dram_tensor("wk", [D, HG], BF16, kind="ExternalInput")
    wv = nc.dram_tensor("wv", [D, HG], BF16, kind="ExternalInput")
    wo = nc.dram_tensor("wo", [D, D], BF16, kind="ExternalInput")
    bq = nc.dram_tensor("bq", [HG, 1], F32, kind="ExternalInput")
    bk = nc.dram_tensor("bk", [HG, 1], F32, kind="ExternalInput")
    bv = nc.dram_tensor("bv", [HG, 1], F32, kind="ExternalInput")
    bo_bc = nc.dram_tensor("bo_bc", [128, D], F32, kind="ExternalInput")
    y_out = nc.dram_tensor("y_out", [B * TPC, D], F32, kind="ExternalOutput")

    # AllToAll staging, split per batch-half so the first half's collective
    # overlaps the second half's attention. Core j's tokens in batch b:
    # {j*128..} from the first 1024 and {1024+j*128..} from the second.
    a2a_in = [[nc.dram_tensor(f"a2a_in{b}{hf}", [D, TPC // 2], BF16)
               for hf in range(2)] for b in range(B)]
    a2a_out = [[nc.dram_tensor(f"a2a_out{b}{hf}", [D, TPC // 2], BF16)
                for hf in range(2)] for b in range(B)]

    with TileContext(nc) as tc:
        with tc.tile_pool(name="persist", bufs=1) as pp:
            # qkv weights as [128, kt, 128] bf16
            w_sb = {}
            for name, dram in (("wq", wq), ("wk", wk), ("wv", wv)):
                t = pp.tile([128, KT_D, HG], BF16, name=f"{name}_sb", tag=f"{name}_sb")
                nc.sync.dma_start(out=t[:], in_=dram[:].rearrange("(kt p) n -> p kt n", p=128))
                w_sb[name] = t
            # full Wo as [128, kt, 1024] bf16 (moving operand of stage G);
            # DMA deferred into stage B to keep it off the startup critical path
            wo_sb = pp.tile([128, KT_D, D], BF16, name="wo_sb", tag="wo_sb")
            b_sb = {}
            for name, dram in (("bq", bq), ("bk", bk), ("bv", bv)):
                t = pp.tile([HG, 1], F32, name=f"{name}_sb", tag=f"{name}_sb")
                nc.sync.dma_start(out=t[:], in_=dram[:])
                b_sb[name] = t
            bo_sb = pp.tile([128, D], F32, name="bo_sb", tag="bo_sb")

            # identity for PE transposes, in bf16
            ident_f = pp.tile([128, 128], F32, name="ident_f", tag="ident_f")
            make_identity(nc, ident_f[:])
            ident = pp.tile([128, 128], BF16, name="ident", tag="ident")
            nc.vector.tensor_copy(ident[:], ident_f[:])
            # multiplicative causal mask for diagonal 128x128 tiles of
            # scores^T: tri01[k, q] = 1 where k <= q else 0
            tri_f = pp.tile([128, 128], F32, name="tri_f", tag="tri_f")
            nc.gpsimd.memset(tri_f[:], 1.0)
            nc.gpsimd.affine_select(
                out=tri_f[:], in_=tri_f[:],
                compare_op=mybir.AluOpType.is_ge, fill=0.0,
                base=0, pattern=[[1, 128]], channel_multiplier=-1,
            )
            tri01 = pp.tile([128, 128], BF16, name="tri01", tag="tri01")
            nc.vector.tensor_copy(tri01[:], tri_f[:])
            # ones row (partition 64) for the r-broadcast outer product
            ones_f = pp.tile([65, 128], F32, name="ones_f", tag="ones_f")
            nc.vector.memset(ones_f[:], 1.0)
            ones_r = pp.tile([65, 128], F32R, name="ones_r", tag="ones_r")
            nc.vector.tensor_copy(ones_r[:], ones_f[:])
            ones128 = pp.tile([128, 64], F32, name="ones128", tag="ones128")
            nc.vector.memset(ones128[:], 1.0)

            # per-batch Q^T/K^T/V^T bf16
            qkvT = {}
            for name in ("qT", "kT", "vT"):
                qkvT[name] = [pp.tile([128, S], BF16, name=f"{name}{b}", tag=f"{name}{b}")
                              for b in range(B)]
            # V_aug: [token-part, pair, ktile, dh+1], col 64 = ones
            vaug = pp.tile([128, B * 2, S // 128, DH + 1], BF16, name="vaug", tag="vaug")
            nc.vector.tensor_copy(vaug[:, :, :, DH:DH + 1], ones128[:, :])
            # normalized ctx^T per batch, bf16
            ctxn = [pp.tile([128, S], BF16, name=f"ctxn{b}", tag=f"ctxn{b}")
                    for b in range(B)]

            # ---------------- stage B+C: QKV projections, V_aug ----------------
            def emit_stage_c(b, psT_pool):
                """V^T -> V_aug for batch b (PE transposes + DVE copies)."""
                for h in range(2):
                    pr = b * 2 + h
                    for g in range(2):  # groups of 8 ktiles
                        pst = psT_pool.tile([128, 512], BF16, name="pst",
                                            tag="pst")
                        for j in range(8):
                            kt = g * 8 + j
                            nc.tensor.transpose(
                                out=pst[:, j * DH:(j + 1) * DH],
                                in_=qkvT["vT"][b][h * DH:(h + 1) * DH,
                                                  kt * 128:(kt + 1) * 128],
                                identity=ident[h * DH:(h + 1) * DH,
                                               h * DH:(h + 1) * DH],
                            )
                        nc.vector.tensor_copy(
                            vaug[:, pr, g * 8:(g + 1) * 8, 0:DH],
                            pst[:],
                        )

            # ---------------- stage B+C for batch 0 ----------------
            with (
                tc.tile_pool(name="xt_pool", bufs=4) as xt_pool,
                tc.tile_pool(name="psB", bufs=1, space="PSUM") as psB_pool,
                tc.tile_pool(name="psT", bufs=2, space="PSUM") as psT_pool,
            ):
                for tq in range(2):
                    t0 = tq * 1024
                    ps = [psB_pool.tile([128, 512], F32, name=f"psB{i}",
                                        tag=f"psB{i}") for i in range(6)]
                    for kt in range(KT_D):
                        xt = xt_pool.tile([128, 1024], BF16, name="xt", tag="xt")
                        nc.sync.dma_start(
                            out=xt[:],
                            in_=xT[kt * 128:(kt + 1) * 128, t0:t0 + 1024])
                        for pi, wname in enumerate(("wq", "wk", "wv")):
                            for nch in range(2):
                                nc.tensor.matmul(
                                    ps[pi * 2 + nch][:],
                                    w_sb[wname][:, kt, :],
                                    xt[:, nch * 512:(nch + 1) * 512],
                                    start=(kt == 0), stop=(kt == KT_D - 1),
                                )
                    if tq == 0:
                        # deferred big loads, needed only from stage G on
                        nc.sync.dma_start(
                            out=wo_sb[:],
                            in_=wo[:].rearrange("(kt p) n -> p kt n", p=128))
                        nc.sync.dma_start(out=bo_sb[:], in_=bo_bc[:])
                    # PSUM drains split across ACT and DVE so the next
                    # tq's matmuls aren't gated on a single engine
                    for pi, (dname, bname) in enumerate(
                            (("qT", "bq"), ("kT", "bk"), ("vT", "bv"))):
                        for nch in range(2):
                            dst = qkvT[dname][0][:, t0 + nch * 512:
                                                 t0 + (nch + 1) * 512]
                            if nch == 0:
                                nc.scalar.activation(
                                    out=dst, in_=ps[pi * 2 + nch][:],
                                    func=mybir.ActivationFunctionType.Identity,
                                    bias=b_sb[bname][:, 0:1],
                                )
                            else:
                                nc.vector.tensor_scalar_add(
                                    out=dst,
                                    in0=ps[pi * 2 + nch][:],
                                    scalar1=b_sb[bname][:, 0:1],
                                )
                emit_stage_c(0, psT_pool)

            # ------- stages D-F; batch 1's projections ride inside
            # batch 0's attention to keep the PE dense -------
            with (
                tc.tile_pool(name="psS", bufs=2, space="PSUM") as psS_pool,
                tc.tile_pool(name="psC", bufs=2, space="PSUM") as psC_pool,
                tc.tile_pool(name="psbc", bufs=1, space="PSUM") as psbc_pool,
                tc.tile_pool(name="ex_pool", bufs=4) as ex_pool,
                tc.tile_pool(name="cu_pool", bufs=2) as cu_pool,
            ):
                # deferred normalize/a2a closures, shared across batches so
                # batch 0's last-chunk drain slides under batch 1's start
                pending = []

                def emit_attention_batch(b, tick, chunk_hook=None):
                    pair_idx = [0]
                    for qc in range(S // 512):
                        q0 = qc * 512
                        n_kt = q0 // 128 + 4
                        ps_ctx = {}
                        for h in range(2):
                            ps_ctx[h] = psC_pool.tile([128, 512], F32,
                                                      name="ps_ctx", tag="ps_ctx")
                        for kg in range(n_kt // 2):
                            ka, kb = 2 * kg, 2 * kg + 1
                            offa = max(0, ka * 128 - q0)
                            offb = max(0, kb * 128 - q0)
                            exs = {}
                            for h in range(2):
                                qT_h = qkvT["qT"][b][h * DH:(h + 1) * DH, :]
                                kT_h = qkvT["kT"][b][h * DH:(h + 1) * DH, :]
                                ps_s = psS_pool.tile([128, 1024], F32,
                                                     name="ps_s", tag="ps_s")
                                nc.tensor.matmul(
                                    ps_s[:, offa:512],
                                    kT_h[:, ka * 128:(ka + 1) * 128],
                                    qT_h[:, q0 + offa:q0 + 512],
                                    start=True, stop=True,
                                )
                                nc.tensor.matmul(
                                    ps_s[:, 512 + offb:1024],
                                    kT_h[:, kb * 128:(kb + 1) * 128],
                                    qT_h[:, q0 + offb:q0 + 512],
                                    start=True, stop=True,
                                )
                                ex = ex_pool.tile([128, 1024], BF16, name="ex",
                                                  tag="ex")
                                # one exp over both halves; the gap
                                # [512:512+offb) holds stale-but-finite data
                                # the ctx matmuls never read.
                                nc.scalar.activation(
                                    out=ex[:, offa:1024], in_=ps_s[:, offa:1024],
                                    func=mybir.ActivationFunctionType.Exp,
                                    scale=INV_SCALE,
                                )
                                exs[h] = ex
                            # previous chunk's deferred normalize lands here,
                            # behind this chunk's first scores, so the PE
                            # never idles on the r-chain at chunk boundaries
                            if kg == 0 and pending:
                                for f in pending:
                                    f()
                                pending.clear()
                            # causal mask on diagonal tiles: on GpSimd so it
                            # never queues behind DVE drain work
                            for h in range(2):
                                ex = exs[h]
                                if ka * 128 >= q0:
                                    nc.gpsimd.tensor_mul(
                                        out=ex[:, offa:offa + 128],
                                        in0=ex[:, offa:offa + 128],
                                        in1=tri01[:],
                                    )
                                if kb * 128 >= q0:
                                    nc.gpsimd.tensor_mul(
                                        out=ex[:, 512 + offb:512 + offb + 128],
                                        in0=ex[:, 512 + offb:512 + offb + 128],
                                        in1=tri01[:],
                                    )
                            for h in range(2):
                                pr = b * 2 + h
                                ex = exs[h]
                                nc.tensor.matmul(
                                    ps_ctx[h][0:DH + 1, offa:512],
                                    vaug[:, pr, ka, :],
                                    ex[:, offa:512],
                                    start=(ka == 0), stop=False,
                                    skip_group_check=True,
                                )
                                nc.tensor.matmul(
                                    ps_ctx[h][0:DH + 1, offb:512],
                                    vaug[:, pr, kb, :],
                                    ex[:, 512 + offb:1024],
                                    start=False, stop=(kb == n_kt - 1),
                                    skip_group_check=True,
                                )
                            pair_idx[0] += 1
                            if tick is not None and pair_idx[0] > 2:
                                tick()
                        # ---- stage E: per-chunk normalize. r + the ps_ctx
                        # evacuation run inline (they free the PSUM ring);
                        # the bc broadcast + multiply (and non-final a2a)
                        # are deferred behind the next chunk's scores ----
                        for h in range(2):
                            r_r = cu_pool.tile([65, 512], F32R, name="r_r",
                                               tag="r_r")
                            if qc in (1, 3):
                                ln_f = cu_pool.tile([65, 512], F32, name="ln_f",
                                                    tag="ln_f")
                                nc.scalar.activation(
                                    out=ln_f[64:65, :],
                                    in_=ps_ctx[h][64:65, 0:512],
                                    func=mybir.ActivationFunctionType.Ln)
                                nc.scalar.activation(
                                    out=r_r[64:65, :], in_=ln_f[64:65, :],
                                    func=mybir.ActivationFunctionType.Exp,
                                    scale=-1.0)
                            else:
                                r_t = cu_pool.tile([65, 512], F32, name="r_t",
                                                   tag="r_t")
                                nc.vector.reciprocal(
                                    r_t[64:65, :], ps_ctx[h][64:65, 0:512])
                                nc.vector.tensor_copy(r_r[64:65, :],
                                                      r_t[64:65, :])
                            ctxu = cu_pool.tile([64, 512], F32, name="ctxu",
                                                tag="ctxu")
                            nc.vector.tensor_copy(ctxu[:],
                                                  ps_ctx[h][0:DH, 0:512])

                            def norm_tail(h=h, q0=q0, r_r=r_r, ctxu=ctxu):
                                bc = psbc_pool.tile([64, 512], F32, name="bc",
                                                    tag="bc")
                                nc.tensor.matmul(
                                    bc[0:DH, :],
                                    ones_r[64:65, 0:DH],
                                    r_r[64:65, :],
                                    start=True, stop=True,
                                )
                                nc.vector.tensor_mul(
                                    out=ctxn[b][h * DH:(h + 1) * DH,
                                                q0:q0 + 512],
                                    in0=ctxu[:],
                                    in1=bc[0:DH, :],
                                )

                            pending.append(norm_tail)
                        if qc in (1, 3):
                            def a2a_tail(hf=qc // 2):
                                for j in range(NC):
                                    nc.sync.dma_start(
                                        out=a2a_in[b][hf][j * 128:
                                                          (j + 1) * 128, :],
                                        in_=ctxn[b][:, hf * 1024 + j * 128:
                                                    hf * 1024 + (j + 1) * 128])
                                nc.gpsimd.collective_compute(
                                    "AllToAll",
                                    mybir.AluOpType.bypass,
                                    ins=[a2a_in[b][hf][:]],
                                    outs=[a2a_out[b][hf][:]],
                                    replica_groups=[list(range(NC))],
                                )

                            pending.append(a2a_tail)
                        if b == B - 1 and qc == S // 512 - 1:
                            # last chunk gates the final collective: flush now
                            for f in pending:
                                f()
                            pending.clear()
                        if chunk_hook is not None:
                            chunk_hook(qc)

                with (
                    tc.tile_pool(name="psB1", bufs=1, space="PSUM") as psB1_pool,
                    tc.tile_pool(name="xt1_pool", bufs=16) as xt1_pool,
                ):
                    b1_rounds = [(tq, wname, dname, bname, nch)
                                 for tq in range(2)
                                 for wname, dname, bname in (
                                     ("wq", "qT", "bq"), ("wk", "kT", "bk"),
                                     ("wv", "vT", "bv"))
                                 for nch in range(2)]
                    b1_state = {"next": 0, "xt": {}}

                    def b1_issue_xt(tq):
                        tiles = []
                        for kt in range(KT_D):
                            xt = xt1_pool.tile([128, 1024], BF16, name="xt1",
                                               tag="xt1")
                            nc.sync.dma_start(
                                out=xt[:],
                                in_=xT[kt * 128:(kt + 1) * 128,
                                       S + tq * 1024: S + (tq + 1) * 1024])
                            tiles.append(xt)
                        b1_state["xt"][tq] = tiles

                    def b1_tick():
                        i = b1_state["next"]
                        if i >= len(b1_rounds):
                            return
                        if i == 4:
                            b1_issue_xt(1)  # prefetch tq1 two rounds early
                        b1_state["next"] = i + 1
                        tq, wname, dname, bname, nch = b1_rounds[i]
                        xts = b1_state["xt"][tq]
                        ps1 = psB1_pool.tile([128, 512], F32, name="b1ps",
                                             tag="b1ps")
                        for kt in range(KT_D):
                            nc.tensor.matmul(
                                ps1[:],
                                w_sb[wname][:, kt, :],
                                xts[kt][:, nch * 512:(nch + 1) * 512],
                                start=(kt == 0), stop=(kt == KT_D - 1),
                            )
                        nc.vector.tensor_scalar_add(
                            out=qkvT[dname][1][:, tq * 1024 + nch * 512:
                                               tq * 1024 + (nch + 1) * 512],
                            in0=ps1[:],
                            scalar1=b_sb[bname][:, 0:1],
                        )

                    b1_issue_xt(0)
                    emit_attention_batch(0, b1_tick)
                    # safety: emit any leftover rounds (shouldn't trigger)
                    while b1_state["next"] < len(b1_rounds):
                        b1_tick()
                with tc.tile_pool(name="psT1", bufs=1, space="PSUM") as psT1_pool:
                    emit_stage_c(1, psT1_pool)
                emit_attention_batch(1, None)
            # ---- stage G: all but the final quarter-size collective are
            # already done; G(0,*) and G(1,0) fill the last a2a's window ----
            with (
                tc.tile_pool(name="psG", bufs=2, space="PSUM") as psG_pool,
                tc.tile_pool(name="gx_pool", bufs=1) as gx_pool,
                tc.tile_pool(name="yg_pool", bufs=2) as yg_pool,
            ):
                for b in range(B):
                    for hf in range(2):
                        gx = gx_pool.tile([128, KT_D, TPC // 2], BF16,
                                          name=f"gx{b}{hf}", tag=f"gx{b}{hf}")
                        nc.sync.dma_start(
                            out=gx[:],
                            in_=a2a_out[b][hf][:].rearrange(
                                "(kt p) n -> p kt n", p=128))
                        for oh in range(2):
                            ps_y = psG_pool.tile([128, 512], F32, name="ps_y",
                                                 tag="ps_y")
                            for kt in range(KT_D):
                                nc.tensor.matmul(
                                    ps_y[:],
                                    gx[:, kt, :],
                                    wo_sb[:, kt, oh * 512:(oh + 1) * 512],
                                    start=(kt == 0), stop=(kt == KT_D - 1),
                                )
                            y_sb = yg_pool.tile([128, 512], F32, name="y_sb",
                                                tag="y_sb")
                            nc.vector.tensor_add(
                                out=y_sb[:], in0=ps_y[:],
                                in1=bo_sb[:, oh * 512:(oh + 1) * 512])
                            nc.sync.dma_start(
                                out=y_out[b * TPC + hf * 128:
                                          b * TPC + (hf + 1) * 128,
                                          oh * 512:(oh + 1) * 512],
                                in_=y_sb[:])

    _split_waits(nc)
    return nc


def kernel(x, mask, Wq, bq, Wk, bk, Wv, bv, Wo, bo, trace=False, _in_maps_only=False):
    bf = ml_dtypes.bfloat16
    x = np.asarray(x, dtype=np.float32).reshape(T, D)
    xT = np.ascontiguousarray(x.T).astype(bf)
    Wo_bf = np.ascontiguousarray(np.asarray(Wo, np.float32)).astype(bf)
    bo_bc = np.ascontiguousarray(
        np.broadcast_to(np.asarray(bo, np.float32), (128, D)))
    in_maps = []
    for c in range(NC):
        sl = slice(c * HG, (c + 1) * HG)
        in_maps.append({
            "xT": xT,
            "wq": np.ascontiguousarray(np.asarray(Wq, np.float32)[:, sl]).astype(bf),
            "wk": np.ascontiguousarray(np.asarray(Wk, np.float32)[:, sl]).astype(bf),
            "wv": np.ascontiguousarray(np.asarray(Wv, np.float32)[:, sl]).astype(bf),
            "wo": Wo_bf,
            "bq": np.ascontiguousarray(np.asarray(bq, np.float32)[sl].reshape(HG, 1)),
            "bk": np.ascontiguousarray(np.asarray(bk, np.float32)[sl].reshape(HG, 1)),
            "bv": np.ascontiguousarray(np.asarray(bv, np.float32)[sl].reshape(HG, 1)),
            "bo_bc": bo_bc,
        })
    if _in_maps_only:
        return in_maps
    nc = build_module()
    res = run_bass_kernel_spmd(nc, in_maps, core_ids=list(range(NC)), trace=trace)
    out = np.empty((T, D), dtype=np.float32)
    for c in range(NC):
        yc = res.results[c]["y_out"]  # [2*TPC, D]; rows = (b, half, 128 tokens)
        for b in range(B):
            for hf in range(2):
                out[b * S + hf * 1024 + c * 128:
                    b * S + hf * 1024 + (c + 1) * 128, :] = \
                    yc[b * TPC + hf * 128: b * TPC + (hf + 1) * 128]
    if trace:
        kernel.last_results = res
    return out.reshape(B, S, D)
